# revision 2
# baseline (speedup 1.0000x reference)
"""Additive attention (Bahdanau) fused Trainium2 kernel, data-parallel over batch.

Math: with q = Q @ Wq.T + bq, k = K @ Wk.T + bk,
  scores[b,i,j] = tanh( w_s . (q[b,i] + k[b,j]) + b_s )
                = tanh( qs[b,i] + ks[b,j] + c )
where qs = Q @ (Wq.T @ w_s), ks = K @ (Wk.T @ w_s), c = (bq+bk).w_s + b_s.
The (B,Lq,Lk,H) intermediate is never materialized. tanh bounds scores in
[-1,1], so the softmax needs no max-subtraction; masking is a per-key -120
additive bias on the exp input (exp(-120±1) underflows to exactly 0, same
result as the reference's -1e6 fill). The softmax denominator comes from a
ones-column appended to V inside the attn @ V matmul.

Sharding: batch B=8 across 8 NeuronCores, one batch element per core.
"""

from contextlib import ExitStack

import numpy as np

import concourse.tile as tile
from concourse import bacc, mybir
from concourse.bass import ts
from concourse.bass_utils import run_bass_kernel_spmd
from concourse.masks import make_identity

B, LQ, LK = 8, 512, 512
F = 256          # feature dim of Q/K/V
H = 128          # hidden dim of the additive-attention MLP
P = 128          # SBUF partitions
QT = LQ // P     # query chunks per core
KT = LK // P     # key chunks per core
NCORES = 8
MASK_BIAS = -120.0  # exp(-120 + [-1,1]) == 0.0 in fp32

F32 = mybir.dt.float32

TRACE = False
LAST_RESULT = None


def _emit(tc, d):
    nc = tc.nc
    X = mybir.AxisListType.X
    A = mybir.AluOpType
    AF = mybir.ActivationFunctionType

    with ExitStack() as ctx:
        consts = ctx.enter_context(tc.tile_pool(name="consts", bufs=1))
        big = ctx.enter_context(tc.tile_pool(name="big", bufs=1))
        work = ctx.enter_context(tc.tile_pool(name="work", bufs=2))
        small = ctx.enter_context(tc.tile_pool(name="small", bufs=1))
        st_pool = ctx.enter_context(tc.tile_pool(name="st", bufs=2))
        outp = ctx.enter_context(tc.tile_pool(name="outp", bufs=4))
        ps_setup = ctx.enter_context(tc.tile_pool(name="ps_setup", bufs=2, space="PSUM"))
        ps_qst = ctx.enter_context(tc.tile_pool(name="ps_qst", bufs=1, space="PSUM"))
        ps_bc = ctx.enter_context(tc.tile_pool(name="ps_bc", bufs=1, space="PSUM"))
        ps_acc = ctx.enter_context(tc.tile_pool(name="ps_acc", bufs=1, space="PSUM"))

        # ---------- constants & weight prep ----------
        id128 = consts.tile([P, P], F32)
        make_identity(nc, id128)
        ones_row = consts.tile([1, P], F32)
        nc.vector.memset(ones_row, 1.0)

        wq_sb = consts.tile([H, F], F32)
        nc.sync.dma_start(wq_sb, d["wq"])
        wk_sb = consts.tile([H, F], F32)
        nc.sync.dma_start(wk_sb, d["wk"])
        ws_sb = consts.tile([H, 1], F32)
        nc.sync.dma_start(ws_sb, d["ws"])
        bq_sb = consts.tile([H, 1], F32)
        nc.sync.dma_start(bq_sb, d["bq"])
        bk_sb = consts.tile([H, 1], F32)
        nc.sync.dma_start(bk_sb, d["bk"])
        bs_sb = consts.tile([1, 1], F32)
        nc.sync.dma_start(bs_sb, d["bs"])
        vl_sb = consts.tile([P, 1], F32)
        nc.sync.dma_start(vl_sb, d["vl"])
        iota_sb = consts.tile([P, KT], F32)
        nc.sync.dma_start(iota_sb, d["iota4"])

        # expbias[p, c] = (p + 128c >= valid_len) ? MASK_BIAS : 0
        expbias = consts.tile([P, KT], F32)
        nc.vector.tensor_scalar(expbias, iota_sb, vl_sb, MASK_BIAS, A.is_ge, A.mult)

        bsum = consts.tile([H, 1], F32)
        nc.vector.tensor_tensor(bsum, bq_sb, bk_sb, A.add)

        # u = Wq.T @ w_s, v = Wk.T @ w_s (rows), c = w_s.(bq+bk) + b_s
        u_row_ps = ps_setup.tile([1, F], F32, tag="s")
        nc.tensor.matmul(u_row_ps, ws_sb, wq_sb, start=True, stop=True)
        v_row_ps = ps_setup.tile([1, F], F32, tag="s")
        nc.tensor.matmul(v_row_ps, ws_sb, wk_sb, start=True, stop=True)
        u_row = consts.tile([1, F], F32)
        nc.vector.tensor_copy(u_row, u_row_ps)
        v_row = consts.tile([1, F], F32)
        nc.vector.tensor_copy(v_row, v_row_ps)

        c_ps = ps_setup.tile([1, 1], F32, tag="s")
        nc.tensor.matmul(c_ps, ws_sb, bsum, start=True, stop=True)
        c_sb = consts.tile([1, 1], F32)
        nc.vector.tensor_tensor(c_sb, c_ps, bs_sb, A.add)

        # broadcast u, v to all partitions
        u_bc_ps = ps_setup.tile([P, F], F32, tag="s")
        nc.tensor.matmul(u_bc_ps, ones_row, u_row, start=True, stop=True)
        u_bc = consts.tile([P, F], F32)
        nc.vector.tensor_copy(u_bc, u_bc_ps)
        v_bc_ps = ps_setup.tile([P, F], F32, tag="s")
        nc.tensor.matmul(v_bc_ps, ones_row, v_row, start=True, stop=True)
        v_bc = consts.tile([P, F], F32)
        nc.vector.tensor_copy(v_bc, v_bc_ps)

        # ---------- load Q, K, V ----------
        q_sb = big.tile([P, QT, F], F32)
        nc.sync.dma_start(q_sb, d["queries"].rearrange("(t p) f -> p t f", p=P))
        k_sb = big.tile([P, KT, F], F32)
        nc.sync.dma_start(k_sb, d["keys"].rearrange("(t p) f -> p t f", p=P))
        aug = big.tile([P, KT, F + 1], F32)
        nc.sync.dma_start(aug[:, :, 0:F], d["values"].rearrange("(t p) f -> p t f", p=P))
        nc.vector.memset(aug[:, :, F:F + 1], 1.0)

        # ---------- qs, ks (matvec via DVE mult+reduce) ----------
        qs_pack = small.tile([P, QT], F32)
        ks_pack = small.tile([P, KT], F32)
        for t in range(QT):
            qm = work.tile([P, F], F32, tag="qm")
            nc.vector.tensor_tensor(qm, q_sb[:, t, :], u_bc, A.mult)
            nc.vector.reduce_sum(qs_pack[:, t:t + 1], qm, axis=X)
        for t in range(KT):
            km = work.tile([P, F], F32, tag="km")
            nc.vector.tensor_tensor(km, k_sb[:, t, :], v_bc, A.mult)
            nc.vector.reduce_sum(ks_pack[:, t:t + 1], km, axis=X)

        # qs as a row [1, LQ] (transpose via PE identity matmul), plus c
        qs_row = small.tile([1, LQ], F32)
        for t in range(QT):
            qsT_ps = ps_qst.tile([1, P], F32, tag="qst")
            nc.tensor.matmul(qsT_ps, qs_pack[:, t:t + 1], id128, start=True, stop=True)
            nc.vector.tensor_scalar(qs_row[0:1, ts(t, P)], qsT_ps, c_sb, None, A.add)

        # broadcast qs+c to all partitions: [P, LQ] in PSUM
        qs_bc_ps = ps_bc.tile([P, LQ], F32)
        nc.tensor.matmul(qs_bc_ps, ones_row, qs_row, start=True, stop=True)

        # ---------- scores.T -> exp -> attn.T @ [V | 1] ----------
        accs = [ps_acc.tile([P, F + 1], F32, tag=f"acc{qc}", name=f"acc{qc}")
                for qc in range(QT)]
        for c in range(KT):
            sT = st_pool.tile([P, LQ], F32, tag="sT")
            nc.scalar.activation(sT, qs_bc_ps, AF.Tanh, bias=ks_pack[:, c:c + 1])
            eT = st_pool.tile([P, LQ], F32, tag="eT")
            nc.scalar.activation(eT, sT, AF.Exp, bias=expbias[:, c:c + 1])
            for qc in range(QT):
                nc.tensor.matmul(accs[qc], eT[:, ts(qc, P)], aug[:, c, :],
                                 start=(c == 0), stop=(c == KT - 1))

        # ---------- normalize and store ----------
        for qc in range(QT):
            rec = small.tile([P, 1], F32, tag=f"rec{qc}")
            nc.vector.reciprocal(rec, accs[qc][:, F:F + 1])
            o_sb = outp.tile([P, F], F32, tag="o")
            nc.vector.tensor_scalar(o_sb, accs[qc][:, 0:F], rec, None, A.mult)
            nc.sync.dma_start(d["out"][ts(qc, P), :], o_sb)


_NC = None


def _build_nc():
    nc = bacc.Bacc("TRN2", target_bir_lowering=False, debug=False, num_devices=NCORES)
    d = {}
    d["queries"] = nc.dram_tensor("queries", [LQ, F], F32, kind="ExternalInput").ap()
    d["keys"] = nc.dram_tensor("keys", [LK, F], F32, kind="ExternalInput").ap()
    d["values"] = nc.dram_tensor("values", [LK, F], F32, kind="ExternalInput").ap()
    d["wq"] = nc.dram_tensor("wq", [H, F], F32, kind="ExternalInput").ap()
    d["wk"] = nc.dram_tensor("wk", [H, F], F32, kind="ExternalInput").ap()
    d["ws"] = nc.dram_tensor("ws", [H, 1], F32, kind="ExternalInput").ap()
    d["bq"] = nc.dram_tensor("bq", [H, 1], F32, kind="ExternalInput").ap()
    d["bk"] = nc.dram_tensor("bk", [H, 1], F32, kind="ExternalInput").ap()
    d["bs"] = nc.dram_tensor("bs", [1, 1], F32, kind="ExternalInput").ap()
    d["vl"] = nc.dram_tensor("vl", [P, 1], F32, kind="ExternalInput").ap()
    d["iota4"] = nc.dram_tensor("iota4", [P, KT], F32, kind="ExternalInput").ap()
    d["out"] = nc.dram_tensor("out", [LQ, F], F32, kind="ExternalOutput").ap()

    with tile.TileContext(nc) as tc:
        _emit(tc, d)
    nc.compile()
    return nc


def get_nc():
    global _NC
    if _NC is None:
        _NC = _build_nc()
    return _NC


def make_in_maps(queries, keys, values, valid_lens, Wq, bq, Wk, bk, w_s, b_s):
    f32c = lambda a: np.ascontiguousarray(np.asarray(a), dtype=np.float32)
    qs, ks, vs = f32c(queries), f32c(keys), f32c(values)
    vl = np.asarray(valid_lens)
    iota4 = (np.arange(P, dtype=np.float32)[:, None]
             + P * np.arange(KT, dtype=np.float32)[None, :])
    iota4 = np.ascontiguousarray(iota4)
    shared = {
        "wq": f32c(Wq), "wk": f32c(Wk),
        "ws": f32c(w_s).reshape(H, 1),
        "bq": f32c(bq).reshape(H, 1),
        "bk": f32c(bk).reshape(H, 1),
        "bs": f32c(b_s).reshape(1, 1),
        "iota4": iota4,
    }
    in_maps = []
    for b in range(NCORES):
        m = dict(shared)
        m["queries"] = qs[b]
        m["keys"] = ks[b]
        m["values"] = vs[b]
        m["vl"] = np.full((P, 1), float(vl[b]), dtype=np.float32)
        in_maps.append(m)
    return in_maps


def kernel(queries, keys, values, valid_lens, Wq, bq, Wk, bk, w_s, b_s):
    global LAST_RESULT
    nc = get_nc()
    in_maps = make_in_maps(queries, keys, values, valid_lens, Wq, bq, Wk, bk, w_s, b_s)
    res = run_bass_kernel_spmd(nc, in_maps, list(range(NCORES)), trace=TRACE)
    LAST_RESULT = res
    return np.stack([res.results[b]["out"] for b in range(NCORES)], axis=0)
